# revision 5
# baseline (speedup 1.0000x reference)
"""Bahdanau attention TRN2 kernel.

Reference math (per batch b):
    qe = query @ W1 + b1                       # [Tq, U]
    ve = values @ W2 + b2                      # [Tv, U]
    score[q, v] = sum_u V[u] * tanh(qe[q, u] + ve[v, u])   (+ bV, dropped:
                  softmax over v is shift-invariant, and score itself is not
                  an output)
    attn = softmax(score, axis=v)
    context = attn @ values

Sharding: 8 cores = 4 batches x 2 halves of Tq. Each core handles a
[128, :] slab of queries for one batch; softmax over Tv is fully local.

Per-core dataflow (U=256 lives on partitions as 2 chunks of 128):
  - PE transposes query/values (identity matmul), projects qe_T[u, q] and
    ve_T[u, v]; b1+b2 folded into qe_T via a K=2 ones-matmul.
  - For each group of G queries: DVE tensor_scalar (per-partition scalar
    add, 2x mode) materializes S[u, (q, v)] = ve_T + qe_T[:, q]; one
    large-free-dim ACT Tanh per chunk (in place).
  - V-reduction on PE: per q, two accumulating [K=128, M=1, N=256] matmuls
    with lhsT = V chunk.  tile_position=(0, 32k) col-tiling places score
    rows at PSUM partitions {0,32,64,96}; a 4-bank PSUM tile holds 32 q's
    (4 positions x 8 free slots).  DVE copies the tile to SBUF (DMA cannot
    read PSUM) and an SBUF->SBUF DMA regathers rows into score[q, v].
  - Softmax: ACT Exp with accum_out (fused row-sum), DVE reciprocal +
    tensor_scalar mul.  No max-subtraction: |score| <= ||V||_1 ~ 4.
  - Context: PE transpose of attn, two accumulating matmuls against the
    naturally-laid-out values, copy out of PSUM, DMA to DRAM.
"""

from contextlib import ExitStack

import numpy as np

import concourse.bass as bass
import concourse.tile as tile
from concourse import bacc, masks, mybir
from concourse.bass_utils import run_bass_kernel_spmd

F32 = mybir.dt.float32

N_CORES = 8
B, TQ, TV = 4, 256, 256
H, D, U = 512, 512, 256
TQS = TQ // 2          # 128 queries per core
P = 128
UC = U // P            # 2 partition chunks of U
G = 16                 # queries per tanh group
NG = TQS // G
QPB = 32               # queries per PSUM score tile (4 positions x 8 slots)

_PROGRAM = None


def _build_program():
    nc = bacc.Bacc("TRN2", target_bir_lowering=False, debug=False,
                   num_devices=N_CORES)

    q_in = nc.declare_dram_parameter("q", [TQS, H], F32, isOutput=False)
    v_in = nc.declare_dram_parameter("v", [TV, D], F32, isOutput=False)
    w1_in = nc.declare_dram_parameter("w1", [H, U], F32, isOutput=False)
    w2_in = nc.declare_dram_parameter("w2", [D, U], F32, isOutput=False)
    b1_in = nc.declare_dram_parameter("b1", [U], F32, isOutput=False)
    b2_in = nc.declare_dram_parameter("b2", [U], F32, isOutput=False)
    vv_in = nc.declare_dram_parameter("vv", [U], F32, isOutput=False)
    ctx_out = nc.declare_dram_parameter("ctx", [TQS, D], F32, isOutput=True)
    attn_out = nc.declare_dram_parameter("attn", [TQS, TV], F32, isOutput=True)

    with tile.TileContext(nc) as tc, ExitStack() as octx:
        consts = octx.enter_context(tc.tile_pool(name="consts", bufs=1))
        work = octx.enter_context(tc.tile_pool(name="work", bufs=1))
        s_pool = octx.enter_context(tc.tile_pool(name="s", bufs=2))
        stage_pool = octx.enter_context(tc.tile_pool(name="stage", bufs=2))

        # ---- constants / inputs in SBUF ----
        ident = consts.tile([P, P], F32, name="ident", tag="ident")
        masks.make_identity(nc, ident[:])

        ones2 = consts.tile([2, P], F32, name="ones2", tag="ones2")
        nc.gpsimd.memset(ones2[:], 1.0)

        w1_sb = consts.tile([P, 4 * U], F32, name="w1", tag="w1")
        w2_sb = consts.tile([P, 4 * U], F32, name="w2", tag="w2")
        for k in range(4):
            nc.sync.dma_start(w1_sb[:, k * U:(k + 1) * U], w1_in[k * P:(k + 1) * P, :])
            nc.sync.dma_start(w2_sb[:, k * U:(k + 1) * U], w2_in[k * P:(k + 1) * P, :])

        b12 = consts.tile([2, U], F32, name="b12", tag="b12")
        nc.sync.dma_start(b12[0:1, :], b1_in[:])
        nc.sync.dma_start(b12[1:2, :], b2_in[:])

        v_col = consts.tile([P, UC], F32, name="vcol", tag="vcol")
        for c in range(UC):
            nc.sync.dma_start(v_col[:, c:c + 1], vv_in[c * P:(c + 1) * P])

        query_sb = consts.tile([P, H], F32, name="query", tag="query")
        nc.sync.dma_start(query_sb[:], q_in[:])
        values_sb = [consts.tile([P, D], F32, name=f"values{a}", tag=f"values{a}")
                     for a in range(2)]
        for a in range(2):
            nc.sync.dma_start(values_sb[a][:], v_in[a * P:(a + 1) * P, :])

        qe = [consts.tile([P, TQS], F32, name=f"qe{c}", tag=f"qe{c}") for c in range(UC)]
        ve = [consts.tile([P, TV], F32, name=f"ve{c}", tag=f"ve{c}") for c in range(UC)]

        with tc.tile_pool(name="ph1_ps", bufs=2, space="PSUM") as ph1_ps:
            # ---- transposes: qT[h, q], vT[d, v] ----
            qT = consts.tile([P, H], F32, name="qT", tag="qT")
            for j in range(4):
                ps = ph1_ps.tile([P, P], F32, name="tp", tag="tp")
                nc.tensor.transpose(ps[:], query_sb[:, j * P:(j + 1) * P], ident[:])
                nc.scalar.copy(qT[:, j * P:(j + 1) * P], ps[:])

            vT = consts.tile([P, 4 * TV], F32, name="vT", tag="vT")
            for j in range(4):
                for a in range(2):
                    ps = ph1_ps.tile([P, P], F32, name="tp", tag="tp")
                    nc.tensor.transpose(ps[:], values_sb[a][:, j * P:(j + 1) * P],
                                        ident[:])
                    nc.scalar.copy(vT[:, j * TV + a * P:j * TV + (a + 1) * P], ps[:])

            # ---- projections: qe_T[u, q] (+b1+b2), ve_T[u, v] ----
            for c in range(UC):
                ps = ph1_ps.tile([P, TQS], F32, name="qe_ps", tag="qe_ps")
                for k in range(4):
                    nc.tensor.matmul(ps[:],
                                     w1_sb[:, k * U + c * P:k * U + (c + 1) * P],
                                     qT[:, k * P:(k + 1) * P],
                                     start=(k == 0), stop=False)
                nc.tensor.matmul(ps[:], b12[0:2, c * P:(c + 1) * P], ones2[0:2, :],
                                 start=False, stop=True)
                nc.scalar.copy(qe[c][:], ps[:])

                ps2 = ph1_ps.tile([P, TV], F32, name="ve_ps", tag="ve_ps")
                for k in range(4):
                    nc.tensor.matmul(ps2[:],
                                     w2_sb[:, k * U + c * P:k * U + (c + 1) * P],
                                     vT[:, k * TV:(k + 1) * TV],
                                     start=(k == 0), stop=(k == 3))
                nc.scalar.copy(ve[c][:], ps2[:])

        # ---- main loop: tanh groups + V-reduction ----
        score_sb = work.tile([P, TV], F32, name="score", tag="score")
        with tc.tile_pool(name="score_ps", bufs=2, space="PSUM") as score_ps_pool:
            sps = None
            for g in range(NG):
                s_t = [s_pool.tile([P, G * TV], F32, name=f"s{c}", tag=f"s{c}")
                       for c in range(UC)]
                for i in range(G):
                    q = g * G + i
                    for c in range(UC):
                        nc.vector.tensor_scalar_add(
                            s_t[c][:, i * TV:(i + 1) * TV], ve[c][:],
                            qe[c][:, q:q + 1])
                for c in range(UC):
                    nc.scalar.activation(s_t[c][:], s_t[c][:],
                                         mybir.ActivationFunctionType.Tanh)
                if g % 2 == 0:
                    sps = score_ps_pool.tile([P, 2048], F32, name="sps", tag="sps")
                for i in range(G):
                    w = (g % 2) * G + i          # index within the 32-q psum tile
                    pos, slot = 32 * (w // 8), w % 8
                    for c in range(UC):
                        nc.tensor.matmul(
                            sps[pos:pos + 1, slot * TV:(slot + 1) * TV],
                            v_col[:, c:c + 1], s_t[c][:, i * TV:(i + 1) * TV],
                            start=(c == 0), stop=(c == UC - 1),
                            tile_position=(0, pos))
                if g % 2 == 1:
                    # PSUM -> SBUF (full-tile copy; cost is free-dim bound),
                    # then SBUF -> SBUF DMA regathers rows {0,32,64,96} x 8
                    # slots into 32 consecutive score rows.
                    stg = stage_pool.tile([P, 2048], F32, name="stg", tag="stg")
                    nc.vector.tensor_copy(stg[:], sps[:])
                    row0 = (g - 1) * G
                    src = stg[:].rearrange("(a b) f -> a b f", b=32)[:, 0, :]
                    nc.sync.dma_start(score_sb[row0:row0 + QPB, :], src)

        # ---- softmax ----
        escore = work.tile([P, TV], F32, name="escore", tag="escore")
        rowsum = work.tile([P, 1], F32, name="rowsum", tag="rowsum")
        nc.scalar.activation(escore[:], score_sb[:],
                             mybir.ActivationFunctionType.Exp,
                             accum_out=rowsum[:])
        rinv = work.tile([P, 1], F32, name="rinv", tag="rinv")
        nc.vector.reciprocal(rinv[:], rowsum[:])
        attn_sb = work.tile([P, TV], F32, name="attn", tag="attn")
        nc.vector.tensor_scalar_mul(attn_sb[:], escore[:], rinv[:])
        nc.sync.dma_start(attn_out[:], attn_sb[:])

        # ---- context = attn @ values ----
        with tc.tile_pool(name="tail_ps", bufs=1, space="PSUM") as tail_ps:
            attnT = [work.tile([P, P], F32, name=f"attnT{a}", tag=f"attnT{a}")
                     for a in range(2)]
            for a in range(2):
                ps = tail_ps.tile([P, P], F32, name="tp2", tag=f"tp2_{a}")
                nc.tensor.transpose(ps[:], attn_sb[:, a * P:(a + 1) * P], ident[:])
                nc.vector.tensor_copy(attnT[a][:], ps[:])
            ctx_ps = tail_ps.tile([P, D], F32, name="ctx_ps", tag="ctx_ps")
            for a in range(2):
                nc.tensor.matmul(ctx_ps[:], attnT[a][:], values_sb[a][:],
                                 start=(a == 0), stop=(a == 1))
            ctx_sb = work.tile([P, D], F32, name="ctx_sb", tag="ctx_sb")
            nc.vector.tensor_copy(ctx_sb[:], ctx_ps[:])
            nc.sync.dma_start(ctx_out[:], ctx_sb[:])

    nc.finalize()
    return nc


def _get_program():
    global _PROGRAM
    if _PROGRAM is None:
        _PROGRAM = _build_program()
    return _PROGRAM


TRACE = False
RUN_KWARGS = {}
LAST_RESULT = None


def kernel(query, values, W1, b1, W2, b2, V, bV):
    global LAST_RESULT
    query = np.ascontiguousarray(np.asarray(query, dtype=np.float32))
    values = np.ascontiguousarray(np.asarray(values, dtype=np.float32))
    W1 = np.ascontiguousarray(np.asarray(W1, dtype=np.float32))
    W2 = np.ascontiguousarray(np.asarray(W2, dtype=np.float32))
    b1 = np.ascontiguousarray(np.asarray(b1, dtype=np.float32))
    b2 = np.ascontiguousarray(np.asarray(b2, dtype=np.float32))
    vv = np.ascontiguousarray(np.asarray(V, dtype=np.float32).reshape(U))
    # bV shifts every score equally; softmax is shift-invariant and score is
    # not returned, so it has no effect on either output.

    nc = _get_program()
    in_maps = []
    for core in range(N_CORES):
        b, half = divmod(core, 2)
        in_maps.append({
            "q": np.ascontiguousarray(query[b, half * TQS:(half + 1) * TQS, :]),
            "v": values[b],
            "w1": W1, "w2": W2, "b1": b1, "b2": b2, "vv": vv,
        })

    res = run_bass_kernel_spmd(nc, in_maps, list(range(N_CORES)), trace=TRACE,
                               **RUN_KWARGS)
    LAST_RESULT = res

    context = np.empty((B, TQ, D), dtype=np.float32)
    attn = np.empty((B, TQ, TV, 1), dtype=np.float32)
    for core in range(N_CORES):
        b, half = divmod(core, 2)
        sl = slice(half * TQS, (half + 1) * TQS)
        context[b, sl, :] = res.results[core]["ctx"]
        attn[b, sl, :, 0] = res.results[core]["attn"]
    return context, attn


# revision 6
# speedup vs baseline: 1.4455x; 1.4455x over previous
"""Bahdanau attention TRN2 kernel.

Reference math (per batch b):
    qe = query @ W1 + b1                       # [Tq, U]
    ve = values @ W2 + b2                      # [Tv, U]
    score[q, v] = sum_u V[u] * tanh(qe[q, u] + ve[v, u])   (+ bV, dropped:
                  softmax over v is shift-invariant, and score itself is not
                  an output)
    attn = softmax(score, axis=v)
    context = attn @ values

Sharding: 8 cores = 4 batches x 2 halves of Tq; softmax over Tv is local.

Per-core dataflow (U=256 on partitions as 2 chunks of 128):
  - PE transposes query/values (identity matmul), projects qe_T[u, q] and
    ve_T[u, v]; b1+b2 folded into qe_T via a K=2 ones-matmul (fp32).
  - Broadcast-add + tanh, engine-balanced per measured rates (DVE
    tensor_scalar 263ns/op, ACT grouped tanh 231ns/q-chunk, ACT direct
    tanh+bias 491ns/op): most q's are staged by DVE into S then tanh'd in
    one large-free-dim ACT op; DIRECT_Q q's per group go straight through
    ACT's fused bias+tanh; optional GPS_Q q's use GPSIMD tensor_tensor
    with a broadcast operand.  tanh output H is MM_DTYPE (bf16 by default)
    for the PE reduction.
  - V-reduction on PE: [K=128, M=1, N=512] matmuls (2 queries per
    matmul), accumulated over the 2 U-chunks into PSUM.  tile_position
    col-tiling spreads rows over partitions {0,32,64,96} x 8 free slots of
    a 4-bank PSUM tile (32 q's each); a full-tile copy (DVE/ACT) moves it
    to SBUF and an SBUF->SBUF DMA regathers score[q, v].
  - Softmax: ACT Exp with fused accum_out row-sum, DVE reciprocal +
    tensor_scalar mul.  No max-subtraction: |score| <= ||V||_1 ~ 4.
  - Context: PE transpose of attn, two fp32 matmuls against values.
"""

from contextlib import ExitStack

import numpy as np

import concourse.bass as bass
import concourse.tile as tile
from concourse import bacc, masks, mybir
from concourse.bass_utils import run_bass_kernel_spmd

F32 = mybir.dt.float32
BF16 = mybir.dt.bfloat16

N_CORES = 8
B, TQ, TV = 4, 256, 256
H, D, U = 512, 512, 256
TQS = TQ // 2          # 128 queries per core
P = 128
UC = U // P            # 2 partition chunks of U
G = 16                 # queries per tanh group
NG = TQS // G

# engine-balance tunables
DIRECT_Q = 2           # q's per group routed via ACT fused bias+tanh
GPS_Q = 0              # q's per group staged by GPSIMD tensor_tensor
STAGE_ON_ACT = 2       # of the 4 psum->sbuf score copies, how many on ACT
MM_DTYPE = "bf16"      # V-reduction dtype: bf16 | f32 | f32r

_PROGRAM = None


def _build_program():
    nc = bacc.Bacc("TRN2", target_bir_lowering=False, debug=False,
                   num_devices=N_CORES)

    q_in = nc.declare_dram_parameter("q", [TQS, H], F32, isOutput=False)
    v_in = nc.declare_dram_parameter("v", [TV, D], F32, isOutput=False)
    w1_in = nc.declare_dram_parameter("w1", [H, U], F32, isOutput=False)
    w2_in = nc.declare_dram_parameter("w2", [D, U], F32, isOutput=False)
    b1_in = nc.declare_dram_parameter("b1", [U], F32, isOutput=False)
    b2_in = nc.declare_dram_parameter("b2", [U], F32, isOutput=False)
    vv_in = nc.declare_dram_parameter("vv", [U], F32, isOutput=False)
    ctx_out = nc.declare_dram_parameter("ctx", [TQS, D], F32, isOutput=True)
    attn_out = nc.declare_dram_parameter("attn", [TQS, TV], F32, isOutput=True)

    h_dt = {"bf16": BF16, "f32": F32, "f32r": F32}[MM_DTYPE]

    def mm_ap(ap):
        return ap.bitcast(mybir.dt.float32r) if MM_DTYPE == "f32r" else ap

    with tile.TileContext(nc) as tc, ExitStack() as octx:
        consts = octx.enter_context(tc.tile_pool(name="consts", bufs=1))
        work = octx.enter_context(tc.tile_pool(name="work", bufs=1))
        s_pool = octx.enter_context(tc.tile_pool(name="s", bufs=2))
        stage_pool = octx.enter_context(tc.tile_pool(name="stage", bufs=2))

        # ---- constants / inputs in SBUF ----
        ident = consts.tile([P, P], F32, name="ident", tag="ident")
        masks.make_identity(nc, ident[:])

        ones2 = consts.tile([2, P], F32, name="ones2", tag="ones2")
        nc.gpsimd.memset(ones2[:], 1.0)

        w1_sb = consts.tile([P, 4 * U], F32, name="w1", tag="w1")
        w2_sb = consts.tile([P, 4 * U], F32, name="w2", tag="w2")
        nc.sync.dma_start(w1_sb[:].rearrange("p (k u) -> p k u", k=4),
                          w1_in[:].rearrange("(k p) u -> p k u", p=P))
        nc.sync.dma_start(w2_sb[:].rearrange("p (k u) -> p k u", k=4),
                          w2_in[:].rearrange("(k p) u -> p k u", p=P))

        b12 = consts.tile([2, U], F32, name="b12", tag="b12")
        nc.sync.dma_start(b12[0:1, :], b1_in[:])
        nc.sync.dma_start(b12[1:2, :], b2_in[:])

        v_col = consts.tile([P, UC], F32, name="vcol", tag="vcol")
        for c in range(UC):
            nc.sync.dma_start(v_col[:, c:c + 1], vv_in[c * P:(c + 1) * P])
        if MM_DTYPE == "bf16":
            v_col_mm = consts.tile([P, UC], BF16, name="vcol_mm", tag="vcol_mm")
            nc.vector.tensor_copy(v_col_mm[:], v_col[:])
        else:
            v_col_mm = v_col

        query_sb = consts.tile([P, H], F32, name="query", tag="query")
        nc.sync.dma_start(query_sb[:], q_in[:])
        values_big = consts.tile([P, 2 * D], F32, name="values", tag="values")
        nc.sync.dma_start(values_big[:].rearrange("p (a d) -> p a d", a=2),
                          v_in[:].rearrange("(a p) d -> p a d", p=P))
        values_sb = [values_big[:, a * D:(a + 1) * D] for a in range(2)]

        qe = [consts.tile([P, TQS], F32, name=f"qe{c}", tag=f"qe{c}")
              for c in range(UC)]
        ve = [consts.tile([P, TV], F32, name=f"ve{c}", tag=f"ve{c}")
              for c in range(UC)]

        with tc.tile_pool(name="ph1_ps", bufs=2, space="PSUM") as ph1_ps:
            # ---- transposes: qT[h, q], vT[d, v] ----
            qT = consts.tile([P, H], F32, name="qT", tag="qT")
            for j in range(4):
                ps = ph1_ps.tile([P, P], F32, name="tp", tag="tp")
                nc.tensor.transpose(ps[:], query_sb[:, j * P:(j + 1) * P], ident[:])
                nc.vector.tensor_copy(qT[:, j * P:(j + 1) * P], ps[:])

            vT = consts.tile([P, 4 * TV], F32, name="vT", tag="vT")
            for j in range(4):
                for a in range(2):
                    ps = ph1_ps.tile([P, P], F32, name="tp", tag="tp")
                    nc.tensor.transpose(ps[:], values_sb[a][:, j * P:(j + 1) * P],
                                        ident[:])
                    nc.vector.tensor_copy(
                        vT[:, j * TV + a * P:j * TV + (a + 1) * P], ps[:])

            # ---- projections: qe_T[u, q] (+b1+b2), ve_T[u, v] ----
            for c in range(UC):
                ps = ph1_ps.tile([P, TQS], F32, name="qe_ps", tag="qe_ps")
                for k in range(4):
                    nc.tensor.matmul(ps[:],
                                     w1_sb[:, k * U + c * P:k * U + (c + 1) * P],
                                     qT[:, k * P:(k + 1) * P],
                                     start=(k == 0), stop=False)
                nc.tensor.matmul(ps[:], b12[0:2, c * P:(c + 1) * P], ones2[0:2, :],
                                 start=False, stop=True)
                nc.vector.tensor_copy(qe[c][:], ps[:])

                ps2 = ph1_ps.tile([P, TV], F32, name="ve_ps", tag="ve_ps")
                for k in range(4):
                    nc.tensor.matmul(ps2[:],
                                     w2_sb[:, k * U + c * P:k * U + (c + 1) * P],
                                     vT[:, k * TV:(k + 1) * TV],
                                     start=(k == 0), stop=(k == 3))
                nc.vector.tensor_copy(ve[c][:], ps2[:])

        # ---- main loop: broadcast-add + tanh + V-reduction ----
        n_grouped = G - DIRECT_Q            # q's staged through S
        score_sb = work.tile([P, TV], F32, name="score", tag="score")
        stage_k = 0
        with tc.tile_pool(name="score_ps", bufs=2, space="PSUM") as score_ps_pool:
            sps = None
            for g in range(NG):
                s_t = [s_pool.tile([P, n_grouped * TV], F32, name=f"s{c}",
                                   tag=f"s{c}") for c in range(UC)]
                h_t = [s_pool.tile([P, G * TV], h_dt, name=f"h{c}",
                                   tag=f"h{c}") for c in range(UC)]
                for i in range(n_grouped):
                    q = g * G + i
                    for c in range(UC):
                        if i < GPS_Q:
                            nc.gpsimd.tensor_tensor(
                                out=s_t[c][:, i * TV:(i + 1) * TV],
                                in0=ve[c][:],
                                in1=qe[c][:, q:q + 1].broadcast_to([P, TV]),
                                op=mybir.AluOpType.add)
                        else:
                            nc.vector.tensor_scalar_add(
                                s_t[c][:, i * TV:(i + 1) * TV], ve[c][:],
                                qe[c][:, q:q + 1])
                for c in range(UC):
                    nc.scalar.activation(h_t[c][:, 0:n_grouped * TV], s_t[c][:],
                                         mybir.ActivationFunctionType.Tanh)
                for i in range(n_grouped, G):
                    q = g * G + i
                    for c in range(UC):
                        nc.scalar.activation(h_t[c][:, i * TV:(i + 1) * TV],
                                             ve[c][:],
                                             mybir.ActivationFunctionType.Tanh,
                                             bias=qe[c][:, q:q + 1])
                if g % 2 == 0:
                    sps = score_ps_pool.tile([P, 2048], F32, name="sps", tag="sps")
                for i in range(0, G, 2):
                    w = (g % 2) * G + i          # index within the 32-q psum tile
                    pos, slot = 32 * (w // 8), w % 8
                    for c in range(UC):
                        nc.tensor.matmul(
                            sps[pos:pos + 1, slot * TV:(slot + 2) * TV],
                            mm_ap(v_col_mm[:, c:c + 1]),
                            mm_ap(h_t[c][:, i * TV:(i + 2) * TV]),
                            start=(c == 0), stop=(c == UC - 1),
                            tile_position=(0, pos))
                if g % 2 == 1:
                    # PSUM -> SBUF (full-tile copy), then SBUF -> SBUF DMA
                    # regathers rows {0,32,64,96} x 8 slots into 32 score rows.
                    stg = stage_pool.tile([P, 2048], F32, name="stg", tag="stg")
                    if stage_k < STAGE_ON_ACT:
                        nc.scalar.copy(stg[:], sps[:])
                    else:
                        nc.vector.tensor_copy(stg[:], sps[:])
                    stage_k += 1
                    row0 = (g - 1) * G
                    src = stg[:].rearrange("(a b) f -> a b f", b=32)[:, 0, :]
                    nc.sync.dma_start(score_sb[row0:row0 + 32, :], src)

        # ---- softmax ----
        escore = work.tile([P, TV], F32, name="escore", tag="escore")
        rowsum = work.tile([P, 1], F32, name="rowsum", tag="rowsum")
        nc.scalar.activation(escore[:], score_sb[:],
                             mybir.ActivationFunctionType.Exp,
                             accum_out=rowsum[:])
        rinv = work.tile([P, 1], F32, name="rinv", tag="rinv")
        nc.vector.reciprocal(rinv[:], rowsum[:])
        attn_sb = work.tile([P, TV], F32, name="attn", tag="attn")
        nc.vector.tensor_scalar_mul(attn_sb[:], escore[:], rinv[:])
        nc.sync.dma_start(attn_out[:], attn_sb[:])

        # ---- context = attn @ values ----
        with tc.tile_pool(name="tail_ps", bufs=1, space="PSUM") as tail_ps:
            attnT = [work.tile([P, P], F32, name=f"attnT{a}", tag=f"attnT{a}")
                     for a in range(2)]
            for a in range(2):
                ps = tail_ps.tile([P, P], F32, name="tp2", tag=f"tp2_{a}")
                nc.tensor.transpose(ps[:], attn_sb[:, a * P:(a + 1) * P], ident[:])
                nc.vector.tensor_copy(attnT[a][:], ps[:])
            ctx_ps = tail_ps.tile([P, D], F32, name="ctx_ps", tag="ctx_ps")
            for a in range(2):
                nc.tensor.matmul(ctx_ps[:], attnT[a][:], values_sb[a],
                                 start=(a == 0), stop=(a == 1))
            ctx_sb = work.tile([P, D], F32, name="ctx_sb", tag="ctx_sb")
            nc.vector.tensor_copy(ctx_sb[:], ctx_ps[:])
            nc.sync.dma_start(ctx_out[:], ctx_sb[:])

    nc.finalize()
    return nc


def _get_program():
    global _PROGRAM
    if _PROGRAM is None:
        _PROGRAM = _build_program()
    return _PROGRAM


TRACE = False
RUN_KWARGS = {}
LAST_RESULT = None


def kernel(query, values, W1, b1, W2, b2, V, bV):
    global LAST_RESULT
    query = np.ascontiguousarray(np.asarray(query, dtype=np.float32))
    values = np.ascontiguousarray(np.asarray(values, dtype=np.float32))
    W1 = np.ascontiguousarray(np.asarray(W1, dtype=np.float32))
    W2 = np.ascontiguousarray(np.asarray(W2, dtype=np.float32))
    b1 = np.ascontiguousarray(np.asarray(b1, dtype=np.float32))
    b2 = np.ascontiguousarray(np.asarray(b2, dtype=np.float32))
    vv = np.ascontiguousarray(np.asarray(V, dtype=np.float32).reshape(U))
    # bV shifts every score equally; softmax is shift-invariant and score is
    # not returned, so it has no effect on either output.

    nc = _get_program()
    in_maps = []
    for core in range(N_CORES):
        b, half = divmod(core, 2)
        in_maps.append({
            "q": np.ascontiguousarray(query[b, half * TQS:(half + 1) * TQS, :]),
            "v": values[b],
            "w1": W1, "w2": W2, "b1": b1, "b2": b2, "vv": vv,
        })

    res = run_bass_kernel_spmd(nc, in_maps, list(range(N_CORES)), trace=TRACE,
                               **RUN_KWARGS)
    LAST_RESULT = res

    context = np.empty((B, TQ, D), dtype=np.float32)
    attn = np.empty((B, TQ, TV, 1), dtype=np.float32)
    for core in range(N_CORES):
        b, half = divmod(core, 2)
        sl = slice(half * TQS, (half + 1) * TQS)
        context[b, sl, :] = res.results[core]["ctx"]
        attn[b, sl, :, 0] = res.results[core]["attn"]
    return context, attn


# revision 7
# speedup vs baseline: 1.4574x; 1.0082x over previous
"""Bahdanau attention TRN2 kernel.

Reference math (per batch b):
    qe = query @ W1 + b1                       # [Tq, U]
    ve = values @ W2 + b2                      # [Tv, U]
    score[q, v] = sum_u V[u] * tanh(qe[q, u] + ve[v, u])   (+ bV, dropped:
                  softmax over v is shift-invariant, and score itself is not
                  an output)
    attn = softmax(score, axis=v)
    context = attn @ values

Sharding: 8 cores = 4 batches x 2 halves of Tq; softmax over Tv is local.

Per-core dataflow (U=256 on partitions as 2 chunks of 128):
  - PE transposes query/values (identity matmul), projects qe_T[u, q] and
    ve_T[u, v]; b1+b2 folded into qe_T via a K=2 ones-matmul (fp32).
  - Broadcast-add + tanh, engine-balanced per measured rates (DVE
    tensor_scalar 263ns/op, ACT grouped tanh 231ns/q-chunk, ACT direct
    tanh+bias 491ns/op): most q's are staged by DVE into S then tanh'd in
    one large-free-dim ACT op; DIRECT_Q q's per group go straight through
    ACT's fused bias+tanh; optional GPS_Q q's use GPSIMD tensor_tensor
    with a broadcast operand.  tanh output H is MM_DTYPE (bf16 by default)
    for the PE reduction.
  - V-reduction on PE: [K=128, M=1, N=512] matmuls (2 queries per
    matmul), accumulated over the 2 U-chunks into PSUM.  tile_position
    col-tiling spreads rows over partitions {0,32,64,96} x 8 free slots of
    a 4-bank PSUM tile (32 q's each); a full-tile copy (DVE/ACT) moves it
    to SBUF and an SBUF->SBUF DMA regathers score[q, v].
  - Softmax: ACT Exp with fused accum_out row-sum, DVE reciprocal +
    tensor_scalar mul.  No max-subtraction: |score| <= ||V||_1 ~ 4.
  - Context: PE transpose of attn, two fp32 matmuls against values.
"""

from contextlib import ExitStack

import numpy as np

import concourse.bass as bass
import concourse.tile as tile
from concourse import bacc, mybir
from concourse.bass_utils import run_bass_kernel_spmd

F32 = mybir.dt.float32
BF16 = mybir.dt.bfloat16

N_CORES = 8
B, TQ, TV = 4, 256, 256
H, D, U = 512, 512, 256
TQS = TQ // 2          # 128 queries per core
P = 128
UC = U // P            # 2 partition chunks of U
G = 8                  # queries per tanh group
NG = TQS // G

# engine-balance tunables
DIRECT_Q = 1           # q's per group routed via ACT fused bias+tanh
GPS_Q = 0              # q's per group staged by GPSIMD tensor_tensor
STAGE_ON_ACT = 2       # of the 4 psum->sbuf score copies, how many on ACT
MM_DTYPE = "bf16"      # V-reduction dtype: bf16 | f32 | f32r

_PROGRAM = None
_IDENT = np.eye(128, dtype=np.float32)
_ONES = np.ones((2, 128), dtype=np.float32)


def _build_program():
    nc = bacc.Bacc("TRN2", target_bir_lowering=False, debug=False,
                   num_devices=N_CORES)

    q_in = nc.declare_dram_parameter("q", [TQS, H], F32, isOutput=False)
    v_in = nc.declare_dram_parameter("v", [TV, D], F32, isOutput=False)
    w1_in = nc.declare_dram_parameter("w1", [H, U], F32, isOutput=False)
    w2_in = nc.declare_dram_parameter("w2", [D, U], F32, isOutput=False)
    b1_in = nc.declare_dram_parameter("b1", [U], F32, isOutput=False)
    b2_in = nc.declare_dram_parameter("b2", [U], F32, isOutput=False)
    vv_in = nc.declare_dram_parameter("vv", [U], F32, isOutput=False)
    ident_in = nc.declare_dram_parameter("ident", [P, P], F32, isOutput=False)
    ones_in = nc.declare_dram_parameter("ones", [2, P], F32, isOutput=False)
    ctx_out = nc.declare_dram_parameter("ctx", [TQS, D], F32, isOutput=True)
    attn_out = nc.declare_dram_parameter("attn", [TQS, TV], F32, isOutput=True)

    h_dt = {"bf16": BF16, "f32": F32, "f32r": F32}[MM_DTYPE]

    def mm_ap(ap):
        return ap.bitcast(mybir.dt.float32r) if MM_DTYPE == "f32r" else ap

    with tile.TileContext(nc) as tc, ExitStack() as octx:
        consts = octx.enter_context(tc.tile_pool(name="consts", bufs=1))
        work = octx.enter_context(tc.tile_pool(name="work", bufs=1))
        s_pool = octx.enter_context(tc.tile_pool(name="s", bufs=2))
        stage_pool = octx.enter_context(tc.tile_pool(name="stage", bufs=2))

        # ---- constants / inputs in SBUF ----
        ident = consts.tile([P, P], F32, name="ident", tag="ident")
        nc.sync.dma_start(ident[:], ident_in[:])

        ones2 = consts.tile([2, P], F32, name="ones2", tag="ones2")
        nc.sync.dma_start(ones2[:], ones_in[:])

        w1_sb = consts.tile([P, 4 * U], F32, name="w1", tag="w1")
        w2_sb = consts.tile([P, 4 * U], F32, name="w2", tag="w2")
        nc.sync.dma_start(w1_sb[:].rearrange("p (k u) -> p k u", k=4),
                          w1_in[:].rearrange("(k p) u -> p k u", p=P))
        nc.sync.dma_start(w2_sb[:].rearrange("p (k u) -> p k u", k=4),
                          w2_in[:].rearrange("(k p) u -> p k u", p=P))

        b12 = consts.tile([2, U], F32, name="b12", tag="b12")
        nc.sync.dma_start(b12[0:1, :], b1_in[:])
        nc.sync.dma_start(b12[1:2, :], b2_in[:])

        v_col = consts.tile([P, UC], F32, name="vcol", tag="vcol")
        for c in range(UC):
            nc.sync.dma_start(v_col[:, c:c + 1], vv_in[c * P:(c + 1) * P])
        if MM_DTYPE == "bf16":
            v_col_mm = consts.tile([P, UC], BF16, name="vcol_mm", tag="vcol_mm")
            nc.vector.tensor_copy(v_col_mm[:], v_col[:])
        else:
            v_col_mm = v_col

        query_sb = consts.tile([P, H], F32, name="query", tag="query")
        nc.sync.dma_start(query_sb[:], q_in[:])
        values_big = consts.tile([P, 2 * D], F32, name="values", tag="values")
        nc.sync.dma_start(values_big[:].rearrange("p (a d) -> p a d", a=2),
                          v_in[:].rearrange("(a p) d -> p a d", p=P))
        values_sb = [values_big[:, a * D:(a + 1) * D] for a in range(2)]

        qe = [consts.tile([P, TQS], F32, name=f"qe{c}", tag=f"qe{c}")
              for c in range(UC)]
        ve = [consts.tile([P, TV], F32, name=f"ve{c}", tag=f"ve{c}")
              for c in range(UC)]

        with tc.tile_pool(name="ph1_ps", bufs=2, space="PSUM") as ph1_ps:
            # ---- transposes: qT[h, q], vT[d, v] ----
            qT = consts.tile([P, H], F32, name="qT", tag="qT")
            for j in range(4):
                ps = ph1_ps.tile([P, P], F32, name="tp", tag="tp")
                nc.tensor.transpose(ps[:], query_sb[:, j * P:(j + 1) * P], ident[:])
                nc.vector.tensor_copy(qT[:, j * P:(j + 1) * P], ps[:])

            vT = consts.tile([P, 4 * TV], F32, name="vT", tag="vT")
            for j in range(4):
                for a in range(2):
                    ps = ph1_ps.tile([P, P], F32, name="tp", tag="tp")
                    nc.tensor.transpose(ps[:], values_sb[a][:, j * P:(j + 1) * P],
                                        ident[:])
                    nc.vector.tensor_copy(
                        vT[:, j * TV + a * P:j * TV + (a + 1) * P], ps[:])

            # ---- projections: qe_T[u, q] (+b1+b2), ve_T[u, v] ----
            for c in range(UC):
                ps = ph1_ps.tile([P, TQS], F32, name="qe_ps", tag="qe_ps")
                for k in range(4):
                    nc.tensor.matmul(ps[:],
                                     w1_sb[:, k * U + c * P:k * U + (c + 1) * P],
                                     qT[:, k * P:(k + 1) * P],
                                     start=(k == 0), stop=False)
                nc.tensor.matmul(ps[:], b12[0:2, c * P:(c + 1) * P], ones2[0:2, :],
                                 start=False, stop=True)
                nc.vector.tensor_copy(qe[c][:], ps[:])

                ps2 = ph1_ps.tile([P, TV], F32, name="ve_ps", tag="ve_ps")
                for k in range(4):
                    nc.tensor.matmul(ps2[:],
                                     w2_sb[:, k * U + c * P:k * U + (c + 1) * P],
                                     vT[:, k * TV:(k + 1) * TV],
                                     start=(k == 0), stop=(k == 3))
                nc.vector.tensor_copy(ve[c][:], ps2[:])

        # ---- main loop: broadcast-add + tanh + V-reduction ----
        n_grouped = G - DIRECT_Q            # q's staged through S
        score_sb = work.tile([P, TV], F32, name="score", tag="score")
        stage_k = 0
        with tc.tile_pool(name="score_ps", bufs=2, space="PSUM") as score_ps_pool:
            sps = None
            for g in range(NG):
                s_t = [s_pool.tile([P, n_grouped * TV], F32, name=f"s{c}",
                                   tag=f"s{c}") for c in range(UC)]
                h_t = [s_pool.tile([P, G * TV], h_dt, name=f"h{c}",
                                   tag=f"h{c}") for c in range(UC)]
                for i in range(n_grouped):
                    q = g * G + i
                    for c in range(UC):
                        if i < GPS_Q:
                            nc.gpsimd.tensor_tensor(
                                out=s_t[c][:, i * TV:(i + 1) * TV],
                                in0=ve[c][:],
                                in1=qe[c][:, q:q + 1].broadcast_to([P, TV]),
                                op=mybir.AluOpType.add)
                        else:
                            nc.vector.tensor_scalar_add(
                                s_t[c][:, i * TV:(i + 1) * TV], ve[c][:],
                                qe[c][:, q:q + 1])
                for c in range(UC):
                    nc.scalar.activation(h_t[c][:, 0:n_grouped * TV], s_t[c][:],
                                         mybir.ActivationFunctionType.Tanh)
                for i in range(n_grouped, G):
                    q = g * G + i
                    for c in range(UC):
                        nc.scalar.activation(h_t[c][:, i * TV:(i + 1) * TV],
                                             ve[c][:],
                                             mybir.ActivationFunctionType.Tanh,
                                             bias=qe[c][:, q:q + 1])
                if g % (32 // G) == 0:
                    sps = score_ps_pool.tile([P, 2048], F32, name="sps", tag="sps")
                for i in range(0, G, 2):
                    w = (g % (32 // G)) * G + i  # index within the 32-q psum tile
                    pos, slot = 32 * (w // 8), w % 8
                    for c in range(UC):
                        nc.tensor.matmul(
                            sps[pos:pos + 1, slot * TV:(slot + 2) * TV],
                            mm_ap(v_col_mm[:, c:c + 1]),
                            mm_ap(h_t[c][:, i * TV:(i + 2) * TV]),
                            start=(c == 0), stop=(c == UC - 1),
                            tile_position=(0, pos))
                if g % (32 // G) == (32 // G) - 1:
                    # PSUM -> SBUF (full-tile copy), then SBUF -> SBUF DMA
                    # regathers rows {0,32,64,96} x 8 slots into 32 score rows.
                    stg = stage_pool.tile([P, 2048], F32, name="stg", tag="stg")
                    if stage_k < STAGE_ON_ACT:
                        nc.scalar.copy(stg[:], sps[:])
                    else:
                        nc.vector.tensor_copy(stg[:], sps[:])
                    stage_k += 1
                    row0 = (g - (32 // G) + 1) * G
                    src = stg[:].rearrange("(a b) f -> a b f", b=32)[:, 0, :]
                    nc.sync.dma_start(score_sb[row0:row0 + 32, :], src)

        # ---- softmax ----
        escore = work.tile([P, TV], F32, name="escore", tag="escore")
        rowsum = work.tile([P, 1], F32, name="rowsum", tag="rowsum")
        nc.scalar.activation(escore[:], score_sb[:],
                             mybir.ActivationFunctionType.Exp,
                             accum_out=rowsum[:])
        rinv = work.tile([P, 1], F32, name="rinv", tag="rinv")
        nc.vector.reciprocal(rinv[:], rowsum[:])
        attn_sb = work.tile([P, TV], F32, name="attn", tag="attn")
        nc.vector.tensor_scalar_mul(attn_sb[:], escore[:], rinv[:])
        nc.sync.dma_start(attn_out[:], attn_sb[:])

        # ---- context = attn @ values ----
        with tc.tile_pool(name="tail_ps", bufs=1, space="PSUM") as tail_ps:
            attnT = [work.tile([P, P], F32, name=f"attnT{a}", tag=f"attnT{a}")
                     for a in range(2)]
            for a in range(2):
                ps = tail_ps.tile([P, P], F32, name="tp2", tag=f"tp2_{a}")
                nc.tensor.transpose(ps[:], attn_sb[:, a * P:(a + 1) * P], ident[:])
                nc.vector.tensor_copy(attnT[a][:], ps[:])
            ctx_ps = tail_ps.tile([P, D], F32, name="ctx_ps", tag="ctx_ps")
            for a in range(2):
                nc.tensor.matmul(ctx_ps[:], attnT[a][:], values_sb[a],
                                 start=(a == 0), stop=(a == 1))
            ctx_sb = work.tile([P, D], F32, name="ctx_sb", tag="ctx_sb")
            nc.vector.tensor_copy(ctx_sb[:], ctx_ps[:])
            nc.sync.dma_start(ctx_out[:], ctx_sb[:])

    nc.finalize()
    return nc


def _get_program():
    global _PROGRAM
    if _PROGRAM is None:
        _PROGRAM = _build_program()
    return _PROGRAM


TRACE = False
RUN_KWARGS = {}
LAST_RESULT = None


def kernel(query, values, W1, b1, W2, b2, V, bV):
    global LAST_RESULT
    query = np.ascontiguousarray(np.asarray(query, dtype=np.float32))
    values = np.ascontiguousarray(np.asarray(values, dtype=np.float32))
    W1 = np.ascontiguousarray(np.asarray(W1, dtype=np.float32))
    W2 = np.ascontiguousarray(np.asarray(W2, dtype=np.float32))
    b1 = np.ascontiguousarray(np.asarray(b1, dtype=np.float32))
    b2 = np.ascontiguousarray(np.asarray(b2, dtype=np.float32))
    vv = np.ascontiguousarray(np.asarray(V, dtype=np.float32).reshape(U))
    # bV shifts every score equally; softmax is shift-invariant and score is
    # not returned, so it has no effect on either output.

    nc = _get_program()
    in_maps = []
    for core in range(N_CORES):
        b, half = divmod(core, 2)
        in_maps.append({
            "q": np.ascontiguousarray(query[b, half * TQS:(half + 1) * TQS, :]),
            "v": values[b],
            "w1": W1, "w2": W2, "b1": b1, "b2": b2, "vv": vv,
            "ident": _IDENT, "ones": _ONES,
        })

    res = run_bass_kernel_spmd(nc, in_maps, list(range(N_CORES)), trace=TRACE,
                               **RUN_KWARGS)
    LAST_RESULT = res

    context = np.empty((B, TQ, D), dtype=np.float32)
    attn = np.empty((B, TQ, TV, 1), dtype=np.float32)
    for core in range(N_CORES):
        b, half = divmod(core, 2)
        sl = slice(half * TQS, (half + 1) * TQS)
        context[b, sl, :] = res.results[core]["ctx"]
        attn[b, sl, :, 0] = res.results[core]["attn"]
    return context, attn
